# revision 75
# baseline (speedup 1.0000x reference)
"""MoE layer (B=4, N=2048, C=1024, F=4096, E=8, top-2) on 8 trn2 NeuronCores.

Sharding (fast path, b1 == b2 == 0 — the benchmarked case): F-parallel
over all experts.  The host computes the (tiny) router and builds ONE
gated, expert-major token stream shared by all cores; core d holds
f-blocks [4d, 4d+4) of EVERY expert's w1/w2 (same 16.8 MB SBUF footprint
as one full expert) and computes the partial FFN
    y_partial = relu(xg @ w1[fslice].T) @ w2[:, fslice].T
for ALL token-expert pairs.  The host sums the 8 partials per token.

Why F-parallel: every core runs the exact same instruction stream over
the exact same segment sizes (the true per-expert token counts padded
to 128), so per-core work is sum_e pad(n_e)/8 instead of max_e pad(n_e)
— the expert load imbalance vanishes instead of padding every core to
the hottest expert's count.  All matmuls in bf16 (1 cycle/row on the
PE); tokens pre-gated on host; y accumulates in fp32 PSUM across the 4
resident f-blocks of each chunk and retires once, in bf16.
"""

import numpy as np

P = 128
C = 1024
F = 4096
E = 8
NFB = 4  # f-blocks per core (32 total / 8 cores)
SCH = 384  # slow-path token chunk: 3 PSUM banks x 2 C-halves + 2 for h
SCHF = 512  # fast-path max token chunk: y^T accumulates in two passes
# of four single-bank [128, 512] fp32 PSUM tiles (+2 banks for h)
NWARM = 14  # PE warm-up matmuls: a contiguous >=3.4us busy window inside
# the block un-throttles the HAM clock gate (1.2 -> 2.4 GHz) BEFORE real
# matmuls start, and the block ends (~12.9us) right as the first token
# and weight DMAs become consumable — any PE gap between warm-up and the
# real stream would restart the HAM's free-running busy window and leave
# the first real matmuls at half clock


def _build(cap: int):
    """Slow fallback (nonzero biases): expert-parallel, fp32 weights."""
    import concourse.mybir as mybir
    from concourse import bacc
    from concourse.tile import TileContext

    f32 = mybir.dt.float32
    f32r = mybir.dt.float32r
    nS = cap // SCH
    nc = bacc.Bacc(None, target_bir_lowering=False)

    xgT = nc.dram_tensor("xgT", [C, cap], f32, kind="ExternalInput")
    w1t = nc.dram_tensor("w1t", [C, F], f32, kind="ExternalInput")
    w2t = nc.dram_tensor("w2t", [F, C], f32, kind="ExternalInput")
    b1r = nc.dram_tensor("b1r", [P, F // P], f32, kind="ExternalInput")
    b2r = nc.dram_tensor("b2r", [P, C], f32, kind="ExternalInput")
    wg = nc.dram_tensor("wg", [P, cap // P], f32, kind="ExternalInput")
    yg = nc.dram_tensor("yg", [cap, C], f32, kind="ExternalOutput")

    w1v = w1t.ap().rearrange("(co ci) f -> ci co f", ci=P)  # [128, 8, F]
    xgv = xgT.ap().rearrange("(co ci) n -> ci co n", ci=P)  # [128, 8, cap]

    with TileContext(nc) as tc:
        with (
            tc.tile_pool(name="consts", bufs=1) as consts,
            tc.tile_pool(name="wpool", bufs=4) as wpool,
            tc.tile_pool(name="xpool", bufs=2) as xpool,
            tc.tile_pool(name="hpool", bufs=3) as hpool,
            tc.tile_pool(name="ypool", bufs=3) as ypool,
            tc.tile_pool(name="psum_h", bufs=2, space="PSUM") as psum_h,
            tc.tile_pool(name="psum_y", bufs=1, space="PSUM") as psum_y,
        ):
            b1_sb = consts.tile([P, F // P], f32)
            nc.sync.dma_start(b1_sb[:], b1r[:, :])
            b2_sb = consts.tile([P, C], f32)
            nc.sync.dma_start(b2_sb[:], b2r[:, :])
            wg_sb = consts.tile([P, cap // P], f32)
            nc.sync.dma_start(wg_sb[:], wg[:, :])

            for s in range(nS):
                xg_s = xpool.tile([P, 8, SCH], f32r, tag="xg")
                nc.sync.dma_start(xg_s[:], xgv[:, :, s * SCH : (s + 1) * SCH].bitcast(f32r))

                yps = [
                    [
                        psum_y.tile(
                            [P, 512], f32, tag=f"y_{t}_{cc}", name=f"y_{t}_{cc}"
                        )
                        for cc in range(2)
                    ]
                    for t in range(3)
                ]

                for f in range(F // P):  # 32
                    w1c = wpool.tile([P, 8, P], f32r, tag="w1c")
                    nc.sync.dma_start(w1c[:], w1v[:, :, f * P : (f + 1) * P].bitcast(f32r))
                    w2c = wpool.tile([P, C], f32r, tag="w2c")
                    nc.sync.dma_start(w2c[:], w2t[f * P : (f + 1) * P, :].bitcast(f32r))

                    hps = psum_h.tile([P, SCH], f32, tag="h")
                    for c in range(8):
                        nc.tensor.matmul(
                            hps[:],
                            lhsT=w1c[:, c, :],
                            rhs=xg_s[:, c, :],
                            start=(c == 0),
                            stop=(c == 7),
                        )
                    hT = hpool.tile([P, SCH], f32r, tag="hT")
                    nc.scalar.activation(
                        hT[:],
                        hps[:],
                        mybir.ActivationFunctionType.Relu,
                        bias=b1_sb[:, f : f + 1],
                        scale=1.0,
                    )
                    for t in range(3):
                        for cc in range(2):
                            nc.tensor.matmul(
                                yps[t][cc][:],
                                lhsT=hT[:, t * P : (t + 1) * P],
                                rhs=w2c[:, cc * 512 : (cc + 1) * 512],
                                start=(f == 0),
                                stop=(f == F // P - 1),
                            )

                for t in range(3):
                    y_sb = ypool.tile([P, C], f32, tag="y_sb")
                    for cc in range(2):
                        sl = slice(cc * 512, (cc + 1) * 512)
                        nc.vector.tensor_add(y_sb[:, sl], yps[t][cc][:], b2_sb[:, sl])
                    yf = ypool.tile([P, C], f32, tag="yf")
                    nc.scalar.mul(yf[:], y_sb[:], wg_sb[:, s * 3 + t : s * 3 + t + 1])
                    nc.sync.dma_start(
                        yg[(s * 3 + t) * P : (s * 3 + t + 1) * P, :], yf[:]
                    )
    nc.compile()
    return nc


def _chunks(m, first=False):
    """Split a token segment into chunks of <= SCHF, exact sizes.

    Chunks below 256 tokens are mm1 LDWEIGHTS-bound on the PE (weight
    load 107 ns/c-block vs matmul ns = cols/2.4), so a short remainder
    is rebalanced with the previous chunk.  first=True starts with a
    256 chunk so the kernel's first matmul only waits on a 524 KB
    token DMA inside the cold-DMA window.
    """
    sizes = [SCHF] * (m // SCHF)
    rem = m - SCHF * len(sizes)
    if rem:
        sizes.append(rem)
    if len(sizes) >= 2 and sizes[-1] < 2 * P:
        move = 2 * P - sizes[-1]
        sizes[-2] -= move
        sizes[-1] += move
    if first and sizes[0] == SCHF:
        sizes = [2 * P, SCHF - 2 * P] + sizes[1:]
    return sizes


def _build_fast(ns: tuple):
    """Fast path (b1 == 0 and b2 == 0): F-parallel over all experts.

    ns[e] = exact token count of expert e (same on all cores; no
    padding anywhere).  Per core inputs:
      xgf [sum(ns)*C]           bf16 gated tokens, expert-major,
                                per-chunk [ci, co, n] tiles
      w1p [E, 128, 4, 8, 128]   bf16 w1[e][fslice].T tiled for mm1 lhsT
      w2p [E, 128, 4, 1024]     bf16 w2[e][:, fslice].T (fi-major)
    output:
      yg  [1024, sum(ns)] bf16  TRANSPOSED partial y (this core's
                                f-slice term); host adds + transposes

    mm2 runs TRANSPOSED: w2 128x128 blocks are the stationary operand
    and hT the moving one, so the token dimension is the free axis —
    no 128-token output-tile quantization, tokens cost exactly what
    they are.  y^T accumulates over the 4 resident f-blocks in eight
    [128, chunk] fp32 PSUM tiles, as two passes of 4 C-blocks (4 PSUM
    banks each + 2 for h <= 8); pass A retires while pass B runs.
    All weights stay resident in SBUF (128 KB/partition).
    """
    import concourse.mybir as mybir
    from concourse import bacc
    from concourse.tile import TileContext

    f32 = mybir.dt.float32
    bf16 = mybir.dt.bfloat16
    Mx = sum(ns)
    nc = bacc.Bacc(None, target_bir_lowering=False)

    xgf = nc.dram_tensor("xgf", [Mx * C], bf16, kind="ExternalInput")
    # weight layouts are partition(=ci/fi)-major per expert so a whole
    # expert loads with ONE dma_start
    w1p = nc.dram_tensor("w1p", [E, P, NFB, 8, P], bf16, kind="ExternalInput")
    w2p = nc.dram_tensor("w2p", [E, P, NFB, C], bf16, kind="ExternalInput")
    # y^T output as [C-half, row-in-block, C-block, token]: one DMA per
    # C-half pass covers all four 128-row blocks (channel c decodes as
    # cc*512 + cg*128 + r -> yg[cc, r, cg, :]; host re-folds)
    yg = nc.dram_tensor("yg", [2, P, 4, Mx], bf16, kind="ExternalOutput")

    with TileContext(nc) as tc:
        with (
            tc.tile_pool(name="warm", bufs=1) as warm,
            tc.tile_pool(name="wpool", bufs=1) as wpool,
            tc.tile_pool(name="xpool", bufs=4) as xpool,
            tc.tile_pool(name="hpool", bufs=8) as hpool,
            tc.tile_pool(name="ypool", bufs=3) as ypool,
            tc.tile_pool(name="psum_h", bufs=2, space="PSUM") as psum_h,
            tc.tile_pool(name="psum_y", bufs=1, space="PSUM") as psum_y,
        ):
            # PE warm-up: dummy matmuls on memset tiles keep the PE busy
            # from t~0 so the HAM clock gate un-throttles (1.2 -> 2.4 GHz)
            # while the first weight/token DMAs are still in flight (DMA
            # cold-start means nothing lands before ~12 us regardless of
            # issue order, so real matmuls cannot start earlier anyway).
            warm_w = warm.tile([P, P], bf16, name="warm_w")
            warm_x = warm.tile([P, SCHF], bf16, name="warm_x")
            nc.vector.memset(warm_w[:], 0.0)
            nc.vector.memset(warm_x[:], 0.0)
            warm_ps = psum_h.tile([P, SCHF], f32, tag="h", name="warm_ps")
            for _ in range(NWARM):
                nc.tensor.matmul(
                    warm_ps[:, :SCH], lhsT=warm_w[:], rhs=warm_x[:, :SCH],
                    start=True, stop=True,
                )

            # (expert, chunk) schedule, expert-major, exact sizes
            sched = []  # (e, token_offset, size)
            xoff = 0
            for e in range(E):
                for sz in _chunks(ns[e], first=(e == 0)):
                    sched.append((e, xoff, sz))
                    xoff += sz

            def load_xg(si):
                e, xoff, nx = sched[si]
                xg_s = xpool.tile([P, 8, nx], bf16, tag="xg", name="xg_s")
                src = xgf[xoff * C : (xoff + nx) * C]
                v = src.rearrange("(ci co n) -> ci co n", ci=P, co=8)
                nc.sync.dma_start(xg_s[:], v)
                return xg_s

            w1g = wpool.tile([P, E, NFB, 8, P], bf16, tag="w1g", name="w1g")
            w2g = wpool.tile([P, E, NFB, C], bf16, tag="w2g", name="w2g")

            loaded1 = [False] * E  # w1 slices issued (whole expert)
            loaded2 = [False] * E  # w2 slices issued (whole expert)

            # one bulk dma_start per expert per weight tensor: each
            # dma_start occupies the in-order issue queue for ~0.6 us, so
            # fewer, bigger loads get the critical early bytes moving
            # sooner (expert 0's w1 is split once so the very first
            # matmul only waits on its first f-block)
            def load_w1(e):
                if not loaded1[e]:
                    loaded1[e] = True
                    nc.sync.dma_start(w1g[:, e], w1p[e])

            def load_w2(e):
                if not loaded2[e]:
                    loaded2[e] = True
                    nc.sync.dma_start(w2g[:, e], w2p[e])

            # DMA issue order matters: all loads drain through ONE
            # in-order hardware queue — interleave token-chunk prefetches
            # with weight loads in consumption order (all 16.8 MB of
            # weights up front starves the token DMAs and stalls the PE
            # ~40 us).  w2 defers past the early window: chunk 0's mm2
            # only starts after all of its mm1.
            xg_q = [load_xg(0)]
            loaded1[0] = True
            nc.sync.dma_start(w1g[:, 0, 0], w1p[0, :, 0])
            nc.sync.dma_start(w1g[:, 0, 1], w1p[0, :, 1])
            nc.sync.dma_start(w1g[:, 0, 2:NFB], w1p[0, :, 2:NFB])
            if len(sched) > 1:
                xg_q.append(load_xg(1))
            load_w2(0)
            if len(sched) > 2:
                xg_q.append(load_xg(2))
            PREF = 3  # xg prefetch depth (xpool bufs = PREF + 1)

            def mm2_unit(u, split_dma=False):
                # transposed mm2: stationary = w2 128x128 block, moving =
                # hT — the free dim is the exact token count, so tokens
                # cost exactly what they are (no 128-row tile rounding)
                e, xoff, nx, cc, fl, hTs, ybox = u
                if fl == 0:
                    # lazy PSUM alloc at first use: 4 single-bank tiles
                    ybox[cc] = [
                        psum_y.tile([P, SCHF], f32, tag=f"y_{c}", name=f"y_{c}")
                        for c in range(4)
                    ]
                yps = ybox[cc]
                hT = hTs[fl]
                last = fl == NFB - 1
                if last:
                    yf = ypool.tile([P, 4, SCHF], bf16, tag=f"yfb{cc}",
                                    name="yfb")
                for cg in range(4):
                    cbase = (cc * 4 + cg) * P
                    nc.tensor.matmul(
                        yps[cg][:, :nx],
                        lhsT=w2g[:, e, fl, cbase : cbase + P],
                        rhs=hT[:, :nx],
                        start=(fl == 0),
                        stop=last,
                    )
                    if last:
                        # copy each C-block as ITS accumulation closes,
                        # alternating DVE / ACT; the PSUM bank frees at
                        # the copy, and ONE wide DMA retires the whole
                        # C-half (dma_start issue slots are ~0.6 us each
                        # on the in-order queue — 8 per chunk head-blocks)
                        if cg % 2 == 0:
                            nc.vector.tensor_copy(
                                yf[:, cg, :nx], yps[cg][:, :nx]
                            )
                        else:
                            nc.scalar.activation(
                                yf[:, cg, :nx], yps[cg][:, :nx],
                                mybir.ActivationFunctionType.Copy,
                            )
                        if split_dma and cg == 1:
                            # tail shave (very last unit only): first half
                            # transfers while the second half still runs
                            nc.sync.dma_start(
                                yg[cc, :, 0:2, xoff : xoff + nx],
                                yf[:, 0:2, :nx],
                            )
                if last:
                    if split_dma:
                        nc.sync.dma_start(
                            yg[cc, :, 2:4, xoff : xoff + nx], yf[:, 2:4, :nx]
                        )
                    else:
                        nc.sync.dma_start(
                            yg[cc, :, :, xoff : xoff + nx], yf[:, :, :nx]
                        )

            # cross-chunk software pipeline: each chunk's eight mm2 units
            # (2 C-halves x 4 f-blocks) are spread across the NEXT chunk's
            # mm1 blocks, so a unit's PSUM-bank turnaround (retire copy +
            # semaphore hops, ~1 us) is covered by ~1.7 us of mm1 work
            # instead of stalling the PE at every pass boundary.  w2 bytes
            # also stay a full chunk behind mm1's — out of the scarce
            # cold-DMA window at kernel start.
            pending = []  # mm2 units: (e, xoff, nx, cc, fl, hTs, ybox)

            for si, (e, xoff, nx) in enumerate(sched):
                if si == 0 or sched[si - 1][0] != e:
                    load_w1(e)  # safety: must be resident now
                    load_w2(e)
                    seg_chunk = 0
                else:
                    seg_chunk += 1
                xg_s = xg_q.pop(0)
                if si + PREF < len(sched):
                    xg_q.append(load_xg(si + PREF))
                if e + 1 < E:
                    # defer the next expert's bulk weight loads to the
                    # segment's 3rd/4th chunks: they aren't consumed for
                    # ~50 us, and issuing them earlier crowds this
                    # segment's own w2/xg bytes out of the (still
                    # ramping, in-order) DMA queue — every segment has
                    # >= 4 chunks, and the segment-entry safety flush
                    # covers degenerate schedules
                    if seg_chunk == 2:
                        load_w1(e + 1)
                    elif seg_chunk == 3:
                        load_w2(e + 1)

                hTs = []
                ybox = [None, None]
                for fl in range(NFB):
                    hps = psum_h.tile([P, SCHF], f32, tag="h", name="hps")
                    for c in range(8):
                        nc.tensor.matmul(
                            hps[:, :nx],
                            lhsT=w1g[:, e, fl, c, :],
                            rhs=xg_s[:, c, :],
                            start=(c == 0),
                            stop=(c == 7),
                        )
                    hT = hpool.tile([P, SCHF], bf16, tag="hT", name="hT")
                    nc.scalar.activation(
                        hT[:, :nx],
                        hps[:, :nx],
                        mybir.ActivationFunctionType.Relu,
                    )
                    hTs.append(hT)
                    for cc in range(2):
                        pending.append((e, xoff, nx, cc, fl, hTs, ybox))
                    # issue two pending units per mm1 block, oldest first —
                    # only units whose relu has already been issued (all of
                    # the previous chunk's, or this chunk's earlier fls).
                    # Chunk 0 issues none: its units (and so the first w2
                    # bytes) wait until after its mm1, keeping the scarce
                    # cold-DMA window for mm1's inputs.
                    k = 0
                    while k < 2 and pending and si > 0:
                        u = pending[0]
                        if u[5] is hTs and u[4] >= fl:
                            break  # own-chunk unit, relu not ready
                        mm2_unit(pending.pop(0))
                        k += 1
            while pending:
                u = pending.pop(0)
                mm2_unit(u, split_dma=not pending)
    nc.compile()
    return nc


_CACHE = {}
_TRACE = False  # test harness sets True to capture an NTFF profile
_LAST_RES = None


def _get_nc(key, builder):
    if key not in _CACHE:
        _CACHE[key] = builder()
    return _CACHE[key]


def _route(x_flat, router_w):
    """Top-2 routing, float64 for stable selection. Returns idx/weights per expert."""
    logits = x_flat.astype(np.float64) @ router_w.astype(np.float64).T
    t = np.exp(logits - logits.max(-1, keepdims=True))
    p = t / t.sum(-1, keepdims=True)
    top2 = np.argsort(-p, axis=-1)[:, :2]
    pv = np.take_along_axis(p, top2, axis=-1)
    wn = pv / (pv.sum(-1, keepdims=True) + 1e-9)
    return top2, wn


def kernel(x, router_w, w1, b1, w2, b2):
    import ml_dtypes
    from concourse.bass_utils import run_bass_kernel_spmd

    bf16 = ml_dtypes.bfloat16
    Bx, Nx, Cx = x.shape
    x_flat = np.ascontiguousarray(x.reshape(-1, Cx))
    T = x_flat.shape[0]

    top2, wn = _route(x_flat, router_w)
    idxs, gates = [], []
    for e in range(E):
        sel = top2 == e
        we = np.where(sel, wn, 0.0).sum(-1)
        idx = np.nonzero(sel.any(-1))[0]
        idxs.append(idx)
        gates.append(we[idx].astype(np.float32))

    fast = bool(np.all(b1 == 0) and np.all(b2 == 0))
    global _LAST_RES

    if not fast:
        cap = max(len(i) for i in idxs)
        cap = ((cap + SCH - 1) // SCH) * SCH
        nc = _get_nc(("slow", cap), lambda: _build(cap))
        in_maps = []
        for e in range(E):
            n_e = len(idxs[e])
            xg = np.zeros((cap, Cx), np.float32)
            xg[:n_e] = x_flat[idxs[e]]
            wg = np.zeros(cap, np.float32)
            wg[:n_e] = gates[e]
            in_maps.append(
                {
                    "xgT": np.ascontiguousarray(xg.T),
                    "w1t": np.ascontiguousarray(w1[e].T),
                    "w2t": np.ascontiguousarray(w2[e].T),
                    "b1r": np.ascontiguousarray(b1[e].reshape(F // P, P).T),
                    "b2r": np.ascontiguousarray(np.broadcast_to(b2[e], (P, Cx))),
                    "wg": np.ascontiguousarray(wg.reshape(cap // P, P).T),
                }
            )
        res = run_bass_kernel_spmd(nc, in_maps, core_ids=list(range(E)), trace=_TRACE)
        _LAST_RES = res
        out = np.zeros((T, Cx), np.float32)
        for e in range(E):
            n_e = len(idxs[e])
            out[idxs[e]] += res.results[e]["yg"][:n_e].astype(np.float32)
        return out.reshape(Bx, Nx, Cx)

    # ---- fast path: F-parallel over all experts ----
    # order experts so the very last chunk is as small as possible (the
    # final retire + output DMA is the kernel's tail)
    order = sorted(
        range(E),
        key=lambda e: (-(_chunks(len(idxs[e]))[-1] if len(idxs[e]) else 0), e),
    )
    ns = tuple(len(idxs[e]) for e in order)
    nc = _get_nc(("fastT", ns), lambda: _build_fast(ns))

    # shared gated token stream, expert-major, per-chunk [ci, co, n] tiles,
    # chunk blocks packed tight (exact sizes, no pad rows)
    blocks = []
    for i, e in enumerate(order):
        xgb = (x_flat[idxs[e]] * gates[e][:, None]).astype(bf16)  # pre-gate
        off = 0
        for sz in _chunks(len(idxs[e]), first=(i == 0)):
            blocks.append(
                np.ascontiguousarray(
                    xgb[off : off + sz].reshape(sz, 8, P).transpose(2, 1, 0)
                ).ravel()
            )
            off += sz
    xgf = np.concatenate(blocks)

    # per-core weight slices: core d holds f-blocks [4d, 4d+4) of every expert
    # w1 tiled:  w1t[e][fb, fo, c, ci] -> lhsT layout [ci, c, fo]
    w1t = [
        w1[e].reshape(F // P, P, 8, P).transpose(0, 3, 2, 1).astype(bf16)
        for e in order
    ]
    w2t = [w2[e].T.reshape(F // P, P, Cx).astype(bf16) for e in order]
    in_maps = []
    for d in range(8):
        fsl = slice(NFB * d, NFB * (d + 1))
        w1pd = np.ascontiguousarray(
            np.stack([w1t[i][fsl].transpose(1, 0, 2, 3) for i in range(E)])
        )
        w2pd = np.ascontiguousarray(
            np.stack([w2t[i][fsl].transpose(1, 0, 2) for i in range(E)])
        )
        in_maps.append({"xgf": xgf, "w1p": w1pd, "w2p": w2pd})

    res = run_bass_kernel_spmd(nc, in_maps, core_ids=list(range(8)), trace=_TRACE)
    _LAST_RES = res

    # host combine: sum the 8 transposed partial-y streams, re-fold the
    # [cc, r, cg, tok] channel layout to [C, tok], scatter-add per expert
    ysum = res.results[0]["yg"].astype(np.float32)
    for d in range(1, 8):
        ysum += res.results[d]["yg"].astype(np.float32)
    Mx = ysum.shape[-1]
    ysum = ysum.transpose(0, 2, 1, 3).reshape(Cx, Mx)  # c = cc*512+cg*128+r
    out = np.zeros((T, Cx), np.float32)
    off = 0
    for i, e in enumerate(order):
        n_e = len(idxs[e])
        out[idxs[e]] += ysum[:, off : off + n_e].T
        off += n_e
    return out.reshape(Bx, Nx, Cx)


# revision 76
# speedup vs baseline: 1.0014x; 1.0014x over previous
"""MoE layer (B=4, N=2048, C=1024, F=4096, E=8, top-2) on 8 trn2 NeuronCores.

Sharding (fast path, b1 == b2 == 0 — the benchmarked case): F-parallel
over all experts.  The host computes the (tiny) router and builds ONE
gated, expert-major token stream shared by all cores; core d holds
f-blocks [4d, 4d+4) of EVERY expert's w1/w2 (same 16.8 MB SBUF footprint
as one full expert) and computes the partial FFN
    y_partial = relu(xg @ w1[fslice].T) @ w2[:, fslice].T
for ALL token-expert pairs.  The host sums the 8 partials per token.

Why F-parallel: every core runs the exact same instruction stream over
the exact same segment sizes (the true per-expert token counts padded
to 128), so per-core work is sum_e pad(n_e)/8 instead of max_e pad(n_e)
— the expert load imbalance vanishes instead of padding every core to
the hottest expert's count.  All matmuls in bf16 (1 cycle/row on the
PE); tokens pre-gated on host; y accumulates in fp32 PSUM across the 4
resident f-blocks of each chunk and retires once, in bf16.
"""

import numpy as np

P = 128
C = 1024
F = 4096
E = 8
NFB = 4  # f-blocks per core (32 total / 8 cores)
SCH = 384  # slow-path token chunk: 3 PSUM banks x 2 C-halves + 2 for h
SCHF = 512  # fast-path max token chunk: y^T accumulates in two passes
# of four single-bank [128, 512] fp32 PSUM tiles (+2 banks for h)
NWARM = 15  # PE warm-up matmuls: a contiguous >=3.4us busy window inside
# the block un-throttles the HAM clock gate (1.2 -> 2.4 GHz) BEFORE real
# matmuls start, and the block ends (~12.9us) right as the first token
# and weight DMAs become consumable — any PE gap between warm-up and the
# real stream would restart the HAM's free-running busy window and leave
# the first real matmuls at half clock


def _build(cap: int):
    """Slow fallback (nonzero biases): expert-parallel, fp32 weights."""
    import concourse.mybir as mybir
    from concourse import bacc
    from concourse.tile import TileContext

    f32 = mybir.dt.float32
    f32r = mybir.dt.float32r
    nS = cap // SCH
    nc = bacc.Bacc(None, target_bir_lowering=False)

    xgT = nc.dram_tensor("xgT", [C, cap], f32, kind="ExternalInput")
    w1t = nc.dram_tensor("w1t", [C, F], f32, kind="ExternalInput")
    w2t = nc.dram_tensor("w2t", [F, C], f32, kind="ExternalInput")
    b1r = nc.dram_tensor("b1r", [P, F // P], f32, kind="ExternalInput")
    b2r = nc.dram_tensor("b2r", [P, C], f32, kind="ExternalInput")
    wg = nc.dram_tensor("wg", [P, cap // P], f32, kind="ExternalInput")
    yg = nc.dram_tensor("yg", [cap, C], f32, kind="ExternalOutput")

    w1v = w1t.ap().rearrange("(co ci) f -> ci co f", ci=P)  # [128, 8, F]
    xgv = xgT.ap().rearrange("(co ci) n -> ci co n", ci=P)  # [128, 8, cap]

    with TileContext(nc) as tc:
        with (
            tc.tile_pool(name="consts", bufs=1) as consts,
            tc.tile_pool(name="wpool", bufs=4) as wpool,
            tc.tile_pool(name="xpool", bufs=2) as xpool,
            tc.tile_pool(name="hpool", bufs=3) as hpool,
            tc.tile_pool(name="ypool", bufs=3) as ypool,
            tc.tile_pool(name="psum_h", bufs=2, space="PSUM") as psum_h,
            tc.tile_pool(name="psum_y", bufs=1, space="PSUM") as psum_y,
        ):
            b1_sb = consts.tile([P, F // P], f32)
            nc.sync.dma_start(b1_sb[:], b1r[:, :])
            b2_sb = consts.tile([P, C], f32)
            nc.sync.dma_start(b2_sb[:], b2r[:, :])
            wg_sb = consts.tile([P, cap // P], f32)
            nc.sync.dma_start(wg_sb[:], wg[:, :])

            for s in range(nS):
                xg_s = xpool.tile([P, 8, SCH], f32r, tag="xg")
                nc.sync.dma_start(xg_s[:], xgv[:, :, s * SCH : (s + 1) * SCH].bitcast(f32r))

                yps = [
                    [
                        psum_y.tile(
                            [P, 512], f32, tag=f"y_{t}_{cc}", name=f"y_{t}_{cc}"
                        )
                        for cc in range(2)
                    ]
                    for t in range(3)
                ]

                for f in range(F // P):  # 32
                    w1c = wpool.tile([P, 8, P], f32r, tag="w1c")
                    nc.sync.dma_start(w1c[:], w1v[:, :, f * P : (f + 1) * P].bitcast(f32r))
                    w2c = wpool.tile([P, C], f32r, tag="w2c")
                    nc.sync.dma_start(w2c[:], w2t[f * P : (f + 1) * P, :].bitcast(f32r))

                    hps = psum_h.tile([P, SCH], f32, tag="h")
                    for c in range(8):
                        nc.tensor.matmul(
                            hps[:],
                            lhsT=w1c[:, c, :],
                            rhs=xg_s[:, c, :],
                            start=(c == 0),
                            stop=(c == 7),
                        )
                    hT = hpool.tile([P, SCH], f32r, tag="hT")
                    nc.scalar.activation(
                        hT[:],
                        hps[:],
                        mybir.ActivationFunctionType.Relu,
                        bias=b1_sb[:, f : f + 1],
                        scale=1.0,
                    )
                    for t in range(3):
                        for cc in range(2):
                            nc.tensor.matmul(
                                yps[t][cc][:],
                                lhsT=hT[:, t * P : (t + 1) * P],
                                rhs=w2c[:, cc * 512 : (cc + 1) * 512],
                                start=(f == 0),
                                stop=(f == F // P - 1),
                            )

                for t in range(3):
                    y_sb = ypool.tile([P, C], f32, tag="y_sb")
                    for cc in range(2):
                        sl = slice(cc * 512, (cc + 1) * 512)
                        nc.vector.tensor_add(y_sb[:, sl], yps[t][cc][:], b2_sb[:, sl])
                    yf = ypool.tile([P, C], f32, tag="yf")
                    nc.scalar.mul(yf[:], y_sb[:], wg_sb[:, s * 3 + t : s * 3 + t + 1])
                    nc.sync.dma_start(
                        yg[(s * 3 + t) * P : (s * 3 + t + 1) * P, :], yf[:]
                    )
    nc.compile()
    return nc


def _chunks(m, first=False):
    """Split a token segment into chunks of <= SCHF, exact sizes.

    Chunks below 256 tokens are mm1 LDWEIGHTS-bound on the PE (weight
    load 107 ns/c-block vs matmul ns = cols/2.4), so a short remainder
    is rebalanced with the previous chunk.  first=True starts with a
    256 chunk so the kernel's first matmul only waits on a 524 KB
    token DMA inside the cold-DMA window.
    """
    sizes = [SCHF] * (m // SCHF)
    rem = m - SCHF * len(sizes)
    if rem:
        sizes.append(rem)
    if len(sizes) >= 2 and sizes[-1] < 2 * P:
        move = 2 * P - sizes[-1]
        sizes[-2] -= move
        sizes[-1] += move
    if first and sizes[0] == SCHF:
        sizes = [2 * P, SCHF - 2 * P] + sizes[1:]
    return sizes


def _build_fast(ns: tuple):
    """Fast path (b1 == 0 and b2 == 0): F-parallel over all experts.

    ns[e] = exact token count of expert e (same on all cores; no
    padding anywhere).  Per core inputs:
      xgf [sum(ns)*C]           bf16 gated tokens, expert-major,
                                per-chunk [ci, co, n] tiles
      w1p [E, 128, 4, 8, 128]   bf16 w1[e][fslice].T tiled for mm1 lhsT
      w2p [E, 128, 4, 1024]     bf16 w2[e][:, fslice].T (fi-major)
    output:
      yg  [1024, sum(ns)] bf16  TRANSPOSED partial y (this core's
                                f-slice term); host adds + transposes

    mm2 runs TRANSPOSED: w2 128x128 blocks are the stationary operand
    and hT the moving one, so the token dimension is the free axis —
    no 128-token output-tile quantization, tokens cost exactly what
    they are.  y^T accumulates over the 4 resident f-blocks in eight
    [128, chunk] fp32 PSUM tiles, as two passes of 4 C-blocks (4 PSUM
    banks each + 2 for h <= 8); pass A retires while pass B runs.
    All weights stay resident in SBUF (128 KB/partition).
    """
    import concourse.mybir as mybir
    from concourse import bacc
    from concourse.tile import TileContext

    f32 = mybir.dt.float32
    bf16 = mybir.dt.bfloat16
    Mx = sum(ns)
    nc = bacc.Bacc(None, target_bir_lowering=False)

    xgf = nc.dram_tensor("xgf", [Mx * C], bf16, kind="ExternalInput")
    # weight layouts are partition(=ci/fi)-major per expert so a whole
    # expert loads with ONE dma_start
    w1p = nc.dram_tensor("w1p", [E, P, NFB, 8, P], bf16, kind="ExternalInput")
    w2p = nc.dram_tensor("w2p", [E, P, NFB, C], bf16, kind="ExternalInput")
    # y^T output as [C-half, row-in-block, C-block, token]: one DMA per
    # C-half pass covers all four 128-row blocks (channel c decodes as
    # cc*512 + cg*128 + r -> yg[cc, r, cg, :]; host re-folds)
    yg = nc.dram_tensor("yg", [2, P, 4, Mx], bf16, kind="ExternalOutput")

    with TileContext(nc) as tc:
        with (
            tc.tile_pool(name="warm", bufs=1) as warm,
            tc.tile_pool(name="wpool", bufs=1) as wpool,
            tc.tile_pool(name="xpool", bufs=4) as xpool,
            tc.tile_pool(name="hpool", bufs=8) as hpool,
            tc.tile_pool(name="ypool", bufs=3) as ypool,
            tc.tile_pool(name="psum_h", bufs=2, space="PSUM") as psum_h,
            tc.tile_pool(name="psum_y", bufs=1, space="PSUM") as psum_y,
        ):
            # PE warm-up: dummy matmuls on memset tiles keep the PE busy
            # from t~0 so the HAM clock gate un-throttles (1.2 -> 2.4 GHz)
            # while the first weight/token DMAs are still in flight (DMA
            # cold-start means nothing lands before ~12 us regardless of
            # issue order, so real matmuls cannot start earlier anyway).
            warm_w = warm.tile([P, P], bf16, name="warm_w")
            warm_x = warm.tile([P, SCHF], bf16, name="warm_x")
            nc.vector.memset(warm_w[:], 0.0)
            nc.vector.memset(warm_x[:], 0.0)
            warm_ps = psum_h.tile([P, SCHF], f32, tag="h", name="warm_ps")
            for _ in range(NWARM):
                nc.tensor.matmul(
                    warm_ps[:, :SCH], lhsT=warm_w[:], rhs=warm_x[:, :SCH],
                    start=True, stop=True,
                )

            # (expert, chunk) schedule, expert-major, exact sizes
            sched = []  # (e, token_offset, size)
            xoff = 0
            for e in range(E):
                for sz in _chunks(ns[e], first=(e == 0)):
                    sched.append((e, xoff, sz))
                    xoff += sz

            def load_xg(si):
                e, xoff, nx = sched[si]
                xg_s = xpool.tile([P, 8, nx], bf16, tag="xg", name="xg_s")
                src = xgf[xoff * C : (xoff + nx) * C]
                v = src.rearrange("(ci co n) -> ci co n", ci=P, co=8)
                nc.sync.dma_start(xg_s[:], v)
                return xg_s

            w1g = wpool.tile([P, E, NFB, 8, P], bf16, tag="w1g", name="w1g")
            w2g = wpool.tile([P, E, NFB, C], bf16, tag="w2g", name="w2g")

            loaded1 = [False] * E  # w1 slices issued (whole expert)
            loaded2 = [False] * E  # w2 slices issued (whole expert)

            # one bulk dma_start per expert per weight tensor: each
            # dma_start occupies the in-order issue queue for ~0.6 us, so
            # fewer, bigger loads get the critical early bytes moving
            # sooner (expert 0's w1 is split once so the very first
            # matmul only waits on its first f-block)
            def load_w1(e):
                if not loaded1[e]:
                    loaded1[e] = True
                    nc.sync.dma_start(w1g[:, e], w1p[e])

            def load_w2(e):
                if not loaded2[e]:
                    loaded2[e] = True
                    nc.sync.dma_start(w2g[:, e], w2p[e])

            # DMA issue order matters: all loads drain through ONE
            # in-order hardware queue — interleave token-chunk prefetches
            # with weight loads in consumption order (all 16.8 MB of
            # weights up front starves the token DMAs and stalls the PE
            # ~40 us).  w2 defers past the early window: chunk 0's mm2
            # only starts after all of its mm1.
            xg_q = [load_xg(0)]
            loaded1[0] = True
            nc.sync.dma_start(w1g[:, 0, 0], w1p[0, :, 0])
            nc.sync.dma_start(w1g[:, 0, 1], w1p[0, :, 1])
            nc.sync.dma_start(w1g[:, 0, 2:NFB], w1p[0, :, 2:NFB])
            if len(sched) > 1:
                xg_q.append(load_xg(1))
            load_w2(0)
            if len(sched) > 2:
                xg_q.append(load_xg(2))
            PREF = 3  # xg prefetch depth (xpool bufs = PREF + 1)

            def mm2_unit(u, split_dma=False):
                # transposed mm2: stationary = w2 128x128 block, moving =
                # hT — the free dim is the exact token count, so tokens
                # cost exactly what they are (no 128-row tile rounding)
                e, xoff, nx, cc, fl, hTs, ybox, final = u
                if fl == 0:
                    # lazy PSUM alloc at first use: 4 single-bank tiles.
                    # The very last unit group has no next-chunk mm1 to
                    # cover its bank turnaround, so it takes the two spare
                    # PSUM banks (y_4/y_5) for its first column groups.
                    tags = ["y_4", "y_5", "y_0", "y_1"] if final else [
                        f"y_{c}" for c in range(4)
                    ]
                    ybox[cc] = [
                        psum_y.tile([P, SCHF], f32, tag=t, name=t)
                        for t in tags
                    ]
                yps = ybox[cc]
                hT = hTs[fl]
                last = fl == NFB - 1
                if last:
                    yf = ypool.tile([P, 4, SCHF], bf16, tag=f"yfb{cc}",
                                    name="yfb")
                for cg in range(4):
                    cbase = (cc * 4 + cg) * P
                    nc.tensor.matmul(
                        yps[cg][:, :nx],
                        lhsT=w2g[:, e, fl, cbase : cbase + P],
                        rhs=hT[:, :nx],
                        start=(fl == 0),
                        stop=last,
                    )
                    if last:
                        # copy each C-block as ITS accumulation closes,
                        # alternating DVE / ACT; the PSUM bank frees at
                        # the copy, and ONE wide DMA retires the whole
                        # C-half (dma_start issue slots are ~0.6 us each
                        # on the in-order queue — 8 per chunk head-blocks)
                        if cg % 2 == 0:
                            nc.vector.tensor_copy(
                                yf[:, cg, :nx], yps[cg][:, :nx]
                            )
                        else:
                            nc.scalar.activation(
                                yf[:, cg, :nx], yps[cg][:, :nx],
                                mybir.ActivationFunctionType.Copy,
                            )
                        if split_dma and cg == 1:
                            # tail shave (very last unit only): first half
                            # transfers while the second half still runs
                            nc.sync.dma_start(
                                yg[cc, :, 0:2, xoff : xoff + nx],
                                yf[:, 0:2, :nx],
                            )
                if last:
                    if split_dma:
                        nc.sync.dma_start(
                            yg[cc, :, 2:4, xoff : xoff + nx], yf[:, 2:4, :nx]
                        )
                    else:
                        nc.sync.dma_start(
                            yg[cc, :, :, xoff : xoff + nx], yf[:, :, :nx]
                        )

            # cross-chunk software pipeline: each chunk's eight mm2 units
            # (2 C-halves x 4 f-blocks) are spread across the NEXT chunk's
            # mm1 blocks, so a unit's PSUM-bank turnaround (retire copy +
            # semaphore hops, ~1 us) is covered by ~1.7 us of mm1 work
            # instead of stalling the PE at every pass boundary.  w2 bytes
            # also stay a full chunk behind mm1's — out of the scarce
            # cold-DMA window at kernel start.
            pending = []  # mm2 units: (e, xoff, nx, cc, fl, hTs, ybox)

            for si, (e, xoff, nx) in enumerate(sched):
                if si == 0 or sched[si - 1][0] != e:
                    load_w1(e)  # safety: must be resident now
                    load_w2(e)
                    seg_chunk = 0
                else:
                    seg_chunk += 1
                xg_s = xg_q.pop(0)
                if si + PREF < len(sched):
                    xg_q.append(load_xg(si + PREF))
                if e + 1 < E:
                    # defer the next expert's bulk weight loads to the
                    # segment's 3rd/4th chunks: they aren't consumed for
                    # ~50 us, and issuing them earlier crowds this
                    # segment's own w2/xg bytes out of the (still
                    # ramping, in-order) DMA queue — every segment has
                    # >= 4 chunks, and the segment-entry safety flush
                    # covers degenerate schedules
                    if seg_chunk == 2:
                        load_w1(e + 1)
                    elif seg_chunk == 3:
                        load_w2(e + 1)

                hTs = []
                ybox = [None, None]
                for fl in range(NFB):
                    hps = psum_h.tile([P, SCHF], f32, tag="h", name="hps")
                    for c in range(8):
                        nc.tensor.matmul(
                            hps[:, :nx],
                            lhsT=w1g[:, e, fl, c, :],
                            rhs=xg_s[:, c, :],
                            start=(c == 0),
                            stop=(c == 7),
                        )
                    hT = hpool.tile([P, SCHF], bf16, tag="hT", name="hT")
                    nc.scalar.activation(
                        hT[:, :nx],
                        hps[:, :nx],
                        mybir.ActivationFunctionType.Relu,
                    )
                    hTs.append(hT)
                    for cc in range(2):
                        pending.append(
                            (e, xoff, nx, cc, fl, hTs, ybox,
                             si == len(sched) - 1 and cc == 1)
                        )
                    # issue two pending units per mm1 block, oldest first —
                    # only units whose relu has already been issued (all of
                    # the previous chunk's, or this chunk's earlier fls).
                    # Chunk 0 issues none: its units (and so the first w2
                    # bytes) wait until after its mm1, keeping the scarce
                    # cold-DMA window for mm1's inputs.
                    k = 0
                    while k < 2 and pending and si > 0:
                        u = pending[0]
                        if u[5] is hTs and u[4] >= fl:
                            break  # own-chunk unit, relu not ready
                        mm2_unit(pending.pop(0))
                        k += 1
            while pending:
                u = pending.pop(0)
                mm2_unit(u, split_dma=not pending)
    nc.compile()
    return nc


_CACHE = {}
_TRACE = False  # test harness sets True to capture an NTFF profile
_LAST_RES = None


def _get_nc(key, builder):
    if key not in _CACHE:
        _CACHE[key] = builder()
    return _CACHE[key]


def _route(x_flat, router_w):
    """Top-2 routing, float64 for stable selection. Returns idx/weights per expert."""
    logits = x_flat.astype(np.float64) @ router_w.astype(np.float64).T
    t = np.exp(logits - logits.max(-1, keepdims=True))
    p = t / t.sum(-1, keepdims=True)
    top2 = np.argsort(-p, axis=-1)[:, :2]
    pv = np.take_along_axis(p, top2, axis=-1)
    wn = pv / (pv.sum(-1, keepdims=True) + 1e-9)
    return top2, wn


def kernel(x, router_w, w1, b1, w2, b2):
    import ml_dtypes
    from concourse.bass_utils import run_bass_kernel_spmd

    bf16 = ml_dtypes.bfloat16
    Bx, Nx, Cx = x.shape
    x_flat = np.ascontiguousarray(x.reshape(-1, Cx))
    T = x_flat.shape[0]

    top2, wn = _route(x_flat, router_w)
    idxs, gates = [], []
    for e in range(E):
        sel = top2 == e
        we = np.where(sel, wn, 0.0).sum(-1)
        idx = np.nonzero(sel.any(-1))[0]
        idxs.append(idx)
        gates.append(we[idx].astype(np.float32))

    fast = bool(np.all(b1 == 0) and np.all(b2 == 0))
    global _LAST_RES

    if not fast:
        cap = max(len(i) for i in idxs)
        cap = ((cap + SCH - 1) // SCH) * SCH
        nc = _get_nc(("slow", cap), lambda: _build(cap))
        in_maps = []
        for e in range(E):
            n_e = len(idxs[e])
            xg = np.zeros((cap, Cx), np.float32)
            xg[:n_e] = x_flat[idxs[e]]
            wg = np.zeros(cap, np.float32)
            wg[:n_e] = gates[e]
            in_maps.append(
                {
                    "xgT": np.ascontiguousarray(xg.T),
                    "w1t": np.ascontiguousarray(w1[e].T),
                    "w2t": np.ascontiguousarray(w2[e].T),
                    "b1r": np.ascontiguousarray(b1[e].reshape(F // P, P).T),
                    "b2r": np.ascontiguousarray(np.broadcast_to(b2[e], (P, Cx))),
                    "wg": np.ascontiguousarray(wg.reshape(cap // P, P).T),
                }
            )
        res = run_bass_kernel_spmd(nc, in_maps, core_ids=list(range(E)), trace=_TRACE)
        _LAST_RES = res
        out = np.zeros((T, Cx), np.float32)
        for e in range(E):
            n_e = len(idxs[e])
            out[idxs[e]] += res.results[e]["yg"][:n_e].astype(np.float32)
        return out.reshape(Bx, Nx, Cx)

    # ---- fast path: F-parallel over all experts ----
    # order experts so the very last chunk is as small as possible (the
    # final retire + output DMA is the kernel's tail)
    order = sorted(
        range(E),
        key=lambda e: (-(_chunks(len(idxs[e]))[-1] if len(idxs[e]) else 0), e),
    )
    ns = tuple(len(idxs[e]) for e in order)
    nc = _get_nc(("fastT", ns), lambda: _build_fast(ns))

    # shared gated token stream, expert-major, per-chunk [ci, co, n] tiles,
    # chunk blocks packed tight (exact sizes, no pad rows)
    blocks = []
    for i, e in enumerate(order):
        xgb = (x_flat[idxs[e]] * gates[e][:, None]).astype(bf16)  # pre-gate
        off = 0
        for sz in _chunks(len(idxs[e]), first=(i == 0)):
            blocks.append(
                np.ascontiguousarray(
                    xgb[off : off + sz].reshape(sz, 8, P).transpose(2, 1, 0)
                ).ravel()
            )
            off += sz
    xgf = np.concatenate(blocks)

    # per-core weight slices: core d holds f-blocks [4d, 4d+4) of every expert
    # w1 tiled:  w1t[e][fb, fo, c, ci] -> lhsT layout [ci, c, fo]
    w1t = [
        w1[e].reshape(F // P, P, 8, P).transpose(0, 3, 2, 1).astype(bf16)
        for e in order
    ]
    w2t = [w2[e].T.reshape(F // P, P, Cx).astype(bf16) for e in order]
    in_maps = []
    for d in range(8):
        fsl = slice(NFB * d, NFB * (d + 1))
        w1pd = np.ascontiguousarray(
            np.stack([w1t[i][fsl].transpose(1, 0, 2, 3) for i in range(E)])
        )
        w2pd = np.ascontiguousarray(
            np.stack([w2t[i][fsl].transpose(1, 0, 2) for i in range(E)])
        )
        in_maps.append({"xgf": xgf, "w1p": w1pd, "w2p": w2pd})

    res = run_bass_kernel_spmd(nc, in_maps, core_ids=list(range(8)), trace=_TRACE)
    _LAST_RES = res

    # host combine: sum the 8 transposed partial-y streams, re-fold the
    # [cc, r, cg, tok] channel layout to [C, tok], scatter-add per expert
    ysum = res.results[0]["yg"].astype(np.float32)
    for d in range(1, 8):
        ysum += res.results[d]["yg"].astype(np.float32)
    Mx = ysum.shape[-1]
    ysum = ysum.transpose(0, 2, 1, 3).reshape(Cx, Mx)  # c = cc*512+cg*128+r
    out = np.zeros((T, Cx), np.float32)
    off = 0
    for i, e in enumerate(order):
        n_e = len(idxs[e])
        out[idxs[e]] += ysum[:, off : off + n_e].T
        off += n_e
    return out.reshape(Bx, Nx, Cx)
